# revision 1
# baseline (speedup 1.0000x reference)
"""StyleGAN-style modulated 3x3 conv on 8 Trainium2 NeuronCores.

Problem: y = conv2d(x, kernel * (style+1) / demod), SAME padding,
  x [B=8, H=128, W=128, C=256], kernel [3,3,C=256,F=256],
  style [B,1,1,C], demod[f] = sqrt(sum_{ky,kx,c} wmod^2 + 1e-8).

Sharding: data-parallel over batch B — each of the 8 cores convolves one
sample with its own modulated/demodulated kernel. No cross-core comm.

Device kernel (per core, all FLOPs on device):
  - modulation: wmod = k * (style+1), DVE tensor_scalar ops pipelined at
    3-tap granularity, output rounded to fp32r (PE 4-byte dtype).
  - demodulation: squares of raw weights accumulated per tap (ACT+DVE),
    scaled by (style+1)^2, channel-summed per f-half via a matmul against
    a ones-column -> invd [f=128, 1] per f-half; applied as a per-partition
    scalar during the PSUM drain, keeping it off the conv critical path.
  - conv as implicit GEMM over PADDED pixel space: output tile = 512
    contiguous padded pixels x 128 f; accumulate 9 taps x 2 c-halves of
    fp32r matmuls  lhsT=wmod[c=128, f=128], moving=x[c=128, 512 px window]
    into PSUM [f=128, 512]; moving windows are single-stride APs (a
    2-level strided AP halves the PE stream rate - measured).  DVE drains
    PSUM * invd -> SBUF; SWDGE DMA out (separate queue set from loads).
  - fp32r: full PE moving rate (measured 1.09 cyc/row) at ~1.5e-4 rel err
    vs the fp32 reference (PE decomposes fp32r into hi/lo bf16 planes).

Host does layout-only marshalling: shard over B, transpose+zero-pad x to
[C, guard + 130*130 + guard] per core (the GEMM needs channels on SBUF
partitions, a halo pad, and guards for the contiguous tap windows),
reshape kernel/style; un-transpose + strip pad columns on gather.
Measured: ~305 us HW exec (8 cores), rel err 1.47e-4 (fp32r), where the
pure-matmul roofline for this shape is ~246 us and the fp32r-rate
roofline is ~276 us.
"""

import sys
import os

for _p in ("/opt/trn_rl_repo", "/root/.axon_site", "/root/.axon_site/_ro/trn_rl_repo",
           "/root/.axon_site/_ro/pypackages"):
    if os.path.isdir(_p) and _p not in sys.path:
        sys.path.append(_p)

import numpy as np

B, H, W, C, F = 8, 128, 128, 256, 256
HP, WP = H + 2, W + 2          # zero-padded image dims (SAME 3x3)
NPIX = HP * WP                 # 16900 padded pixels
CH = C // 128                  # c-half count (contraction is tiled by 128)
NTAP = 9
ROWS_PER_STAGE = 8             # output rows staged per store DMA (1 MB)
N_CORES = 8
NTILE_G = 33                   # 512-px padded output tiles per f-half
GUARD = 132                    # zero guard so tap windows never go OOB
XTLEN = GUARD + 130 + NTILE_G * 512 + 132  # 17290, divisible by 13

_COMPILED = {}


def _build_nc():
    import concourse.bacc as bacc
    import concourse.mybir as mybir
    import concourse.tile as tile

    f32 = mybir.dt.float32
    f32r = mybir.dt.float32r
    AF = mybir.ActivationFunctionType

    nc = bacc.Bacc("TRN2", target_bir_lowering=False, debug=False,
                   num_devices=N_CORES)

    # declared fp32r (same bits as the fp32 host array): the PE does the
    # hi/lo decomposition on load, so no casting DMA is needed
    xt_d = nc.dram_tensor("xt", [CH, 128, XTLEN], f32r, kind="ExternalInput").ap()
    st_d = nc.dram_tensor("st", [128, CH], f32, kind="ExternalInput").ap()
    # weights pre-arranged on host to [c_half, c, tap, f]
    wk_d = nc.dram_tensor("wk", [CH, 128, NTAP, F], f32, kind="ExternalInput").ap()
    # transposed PADDED output: [f_half, f, NTILE*512 padded px starting at
    # padded row 1] (host strips pad cols + re-lays to [H, W, F])
    NTILE = NTILE_G
    yt_d = nc.dram_tensor("yt", [CH, 128, NTILE * 512], f32,
                          kind="ExternalOutput").ap()

    with tile.TileContext(nc) as tc:
        with tc.tile_pool(name="pers", bufs=1) as pers, \
             tc.tile_pool(name="wtmp", bufs=3) as wtmp, \
             tc.tile_pool(name="stage", bufs=8) as stage, \
             tc.tile_pool(name="psum", bufs=6, space="PSUM") as psum_pool, \
             tc.tile_pool(name="psumd", bufs=1, space="PSUM") as psum_d:

            # ---- style: s = style + 1 (one scalar per channel partition) ----
            s_t = pers.tile([128, CH], f32, tag="s", name="s_t")
            nc.sync.dma_start(s_t[:], st_d)
            nc.vector.tensor_scalar_add(s_t[:], s_t[:], 1.0)

            # ---- raw weights + modulation, pipelined at 3-tap granularity
            # so the first conv matmuls start as early as possible ----
            wraw = [pers.tile([128, NTAP, F], f32, tag=f"wraw{ch}", name=f"wraw{ch}")
                    for ch in range(CH)]
            wmod = [pers.tile([128, NTAP, F], f32r, tag=f"wmod{ch}", name=f"wmod{ch}")
                    for ch in range(CH)]
            for t0 in range(0, NTAP, 3):
                for ch in range(CH):
                    nc.sync.dma_start(wraw[ch][:, t0:t0 + 3], wk_d[ch][:, t0:t0 + 3])
                    nc.vector.tensor_scalar_mul(wmod[ch][:, t0:t0 + 3],
                                                wraw[ch][:, t0:t0 + 3],
                                                s_t[:, ch:ch + 1])

            # ---- x image: [c, padded-pix], cast fp32 -> fp32r during DMA ----
            xt = [pers.tile([128, XTLEN], f32r, tag=f"xt{ch}", name=f"xt{ch}")
                  for ch in range(CH)]
            # graded chunks: small first (arrive fast under fair BW sharing,
            # unblocking the first conv groups), large later; c-halves
            # interleaved since each conv group needs both
            bounds = [0]
            pos = 0
            for nrows in [3, 3, 3, 3, 3, 3, 16, 16, 16, 16, 16, 16, 16]:
                pos += nrows * WP
                bounds.append(pos)
            bounds[-1] = XTLEN
            for ck in range(len(bounds) - 1):
                for ch in range(CH):
                    sl = slice(bounds[ck], bounds[ck + 1])
                    nc.gpsimd.dma_start(xt[ch][:, sl], xt_d[ch][:, sl])

            # ---- chain B (drain path): demod reciprocal, f-broadcast ----
            s2_t = pers.tile([128, CH], f32, tag="s2", name="s2_t")
            nc.vector.tensor_mul(out=s2_t[:], in0=s_t[:], in1=s_t[:])
            acc = [pers.tile([128, F], f32, tag=f"acc{ch}", name=f"acc{ch}")
                   for ch in range(CH)]
            for ch in range(CH):
                for t in range(NTAP):
                    sq = wtmp.tile([128, F], f32, tag="sq", name="sq")
                    nc.scalar.activation(sq[:], wraw[ch][:, t], AF.Square)
                    if t == 0:
                        nc.vector.tensor_copy(acc[ch][:], sq[:])
                    else:
                        nc.vector.tensor_add(acc[ch][:], acc[ch][:], sq[:])
                nc.vector.tensor_scalar_mul(acc[ch][:], acc[ch][:],
                                            s2_t[:, ch:ch + 1])
            ones_t = pers.tile([128, 1], f32, tag="ones", name="ones_t")
            nc.vector.memset(ones_t[:], 1.0)
            eps_t = pers.tile([128, 1], f32, tag="eps", name="eps_t")
            nc.vector.memset(eps_t[:], 1e-8)

            def emit_invd():
                # per-f-half demod column: d2fh[f,0] = sum_c acc[c, fh*128+f];
                # emitted AFTER the first conv group so these matmuls sit
                # behind it in the PE queue instead of gating the conv start
                invd_p = []
                for fh in range(CH):
                    d2 = psum_d.tile([128, 1], f32, tag=f"d2_{fh}",
                                     name=f"d2_{fh}")
                    for ch in range(CH):
                        nc.tensor.matmul(d2[:],
                                         acc[ch][:, fh * 128:(fh + 1) * 128],
                                         ones_t[:], start=(ch == 0),
                                         stop=(ch == CH - 1))
                    dm = pers.tile([128, 1], f32, tag=f"dm{fh}", name=f"dm{fh}")
                    nc.scalar.activation(dm[:], d2[:], AF.Sqrt, bias=eps_t[:])
                    iv = pers.tile([128, 1], f32, tag=f"iv{fh}", name=f"iv{fh}")
                    nc.vector.reciprocal(iv[:], dm[:])
                    invd_p.append(iv)
                return invd_p

            invd_p = None
            # ---- main conv: PSUM tile [f=128, 512 contiguous padded px] ----
            # yt[f, j] (padded pos p = 130 + j) =
            #   sum_{ky,kx,c} x_pad[c, p + (ky-1)*WP + kx-1] * w[ky,kx,c,f]
            for g in range(NTILE):
                # valid padded output range ends at 32*512+256: the last
                # tile only needs half its pixels, so halve its streams
                npx = 256 if g == NTILE - 1 else 512
                for fh in range(CH):
                    pt = psum_pool.tile([128, 512], f32, tag="pt", name="pt")
                    i = 0
                    for ky in range(3):
                        for kx in range(3):
                            base = (GUARD + 130 + 512 * g
                                    + (ky - 1) * WP + kx - 1)
                            for ch in range(CH):
                                nc.tensor.matmul(
                                    pt[:, :npx],
                                    wmod[ch][:, 3 * ky + kx,
                                             fh * 128:(fh + 1) * 128],
                                    xt[ch][:, base:base + npx],
                                    start=(i == 0), stop=(i == NTAP * CH - 1))
                                i += 1
                    if invd_p is None:
                        invd_p = emit_invd()
                    # drain applies demodulation: out = psum * invd[f]
                    ot = stage.tile([128, 512], f32, tag="out", name="ot")
                    nc.vector.tensor_scalar_mul(ot[:, :npx], pt[:, :npx],
                                                invd_p[fh][:])
                    nc.gpsimd.dma_start(
                        yt_d[fh][:, g * 512:g * 512 + npx], ot[:, :npx])

    nc.compile()
    return nc


def _get_nc():
    if "nc" not in _COMPILED:
        _COMPILED["nc"] = _build_nc()
    return _COMPILED["nc"]


def _prep_in_maps(x, style, kernel):
    """Host-side layout marshalling: shard over B, transpose+pad x."""
    x = np.ascontiguousarray(x, dtype=np.float32)
    style = np.ascontiguousarray(style, dtype=np.float32)
    kernel = np.ascontiguousarray(kernel, dtype=np.float32)
    # [3,3,C,F] -> [c_half, c_low, tap, f]
    wk = np.ascontiguousarray(
        kernel.reshape(NTAP, CH, 128, F).transpose(1, 2, 0, 3))
    in_maps = []
    for b in range(B):
        xp = np.zeros((C, XTLEN), dtype=np.float32)
        xpv = xp[:, GUARD:GUARD + NPIX].reshape(C, HP, WP)
        xpv[:, 1:H + 1, 1:W + 1] = x[b].transpose(2, 0, 1)
        xt = np.ascontiguousarray(xp.reshape(CH, 128, XTLEN))
        st = np.ascontiguousarray(style[b].reshape(CH, 128).T)
        in_maps.append({"xt": xt, "st": st, "wk": wk})
    return in_maps


def run_cores(x, style, kernel, trace=False, trace_cores=None):
    """Compile (cached) + run on the 8 NeuronCores. Returns (y, results)."""
    from concourse.bass_utils import run_bass_kernel_spmd

    nc = _get_nc()
    in_maps = _prep_in_maps(x, style, kernel)
    kwargs = {}
    if trace:
        kwargs.update(trace=True, trace_cores=trace_cores)
    res = run_bass_kernel_spmd(nc, in_maps, list(range(N_CORES)), **kwargs)
    # yt [f_half, 128, NTILE*512] covers padded positions 130.. ; interior =
    # first 128*WP entries, reshaped [F, H, WP] with pad cols stripped
    y = np.stack(
        [res.results[b]["yt"].reshape(F, -1)[:, :H * WP]
         .reshape(F, H, WP)[:, :, 1:W + 1].transpose(1, 2, 0)
         for b in range(B)], axis=0)
    return y, res


def kernel(x, style, kernel):
    y, _ = run_cores(x, style, kernel)
    return y.astype(np.float32)



# revision 5
# speedup vs baseline: 1.5027x; 1.5027x over previous
"""StyleGAN-style modulated 3x3 conv on 8 Trainium2 NeuronCores.

Problem: y = conv2d(x, kernel * (style+1) / demod), SAME padding,
  x [B=8, H=128, W=128, C=256], kernel [3,3,C=256,F=256],
  style [B,1,1,C], demod[f] = sqrt(sum_{ky,kx,c} wmod^2 + 1e-8).

Sharding: data-parallel over batch B — each of the 8 cores convolves one
sample with its own modulated/demodulated kernel. No cross-core comm.

Algorithm (per core): 1D Winograd F(2,3) along H + direct 3-tap conv
along W, all matmuls in bf16 (PE full rate; rel-err gate is 2e-2, bf16
lands ~3e-3). This cuts PE work to 2/3 of the direct implicit-GEMM
floor: per output row pair, 4 transformed planes replace 6 tap rows.

  - input transform (DVE, bf16 2x): T0=d0-d2, T1=d1+d2, T2=d2-d1,
    T3=d1-d3 where d_k = x_pad[c, 2*ht+k, :] (rows on the free axis,
    full-rate unit-stride inner dim). x streamed in 10-row slots
    (8 new + 2 halo) so x never fully resides in SBUF.
  - weight transform (DVE, bf16): W0=m0, W1=(m0+m1+m2)/2,
    W2=(m0-m1+m2)/2, W3=m2 with m_ky = raw[ky]*(style+1).
  - GEMM (PE): M_j[f,p] += sum_{kx,c} T_j[c, p+kx-1] * W_j[kx,c,f],
    PSUM tile [f=128, 512 px], 24 bf16 matmuls per (group, f-half).
  - output transform fused with demodulation (DVE scalar_tensor_tensor
    with per-partition scalar invd[f]):
      y_even = (M0+M1+M2)*invd, y_odd = (M1-M2-M3)*invd
    via c1s = M1*invd (ACT copy w/ scale), then 4 STT ops; outputs
    stored bf16 (halves store traffic), upcast on host.
  - demod invd from bf16 raw weights: ACT Square(raw*s) then the
    ones-column matmul trick per f-half; emitted after conv group 0
    so it does not gate the conv start.

Host does layout-only marshalling: shard over B, transpose+zero-pad x
to [C, 130, 130] bf16 per core; reshape kernel to [CH,128,9,F] bf16;
un-interleave parity planes + strip pad columns on gather.
"""

import sys
import os

for _p in ("/opt/trn_rl_repo", "/root/.axon_site", "/root/.axon_site/_ro/trn_rl_repo",
           "/root/.axon_site/_ro/pypackages"):
    if os.path.isdir(_p) and _p not in sys.path:
        sys.path.append(_p)

import numpy as np
import ml_dtypes

B, H, W, C, F = 8, 128, 128, 256, 256
CH = C // 128                  # contraction halves
FHN = F // 128                 # f halves
NTAP = 9
WW = W + 2                     # padded width  (w = -1..128)
HP = H + 2                     # padded height (h = -1..128)
NHT = H // 2                   # 64 h-tiles (output row pairs)
TLEN = NHT * WW                # 8320 flat transformed positions
GT = 8                         # zero guard around T planes (+-1 shifts)
NSLOT = 16                     # x slots: 8 fresh rows + 2 halo rows each
SROWS = 10
CHT = 4                        # h-tiles transformed per chunk (= 1 slot)
CLEN = CHT * WW                # 520
NG = 17                        # PE groups per f-half: 16 x 512 + 1 x 128
N_CORES = 8

_COMPILED = {}


def _build_nc():
    import concourse.bacc as bacc
    import concourse.mybir as mybir
    import concourse.tile as tile

    f32 = mybir.dt.float32
    bf16 = mybir.dt.bfloat16
    AF = mybir.ActivationFunctionType
    ALU = mybir.AluOpType

    nc = bacc.Bacc("TRN2", target_bir_lowering=False, debug=False,
                   num_devices=N_CORES)

    xt_d = nc.dram_tensor("xt", [CH, 128, HP * WW], bf16,
                          kind="ExternalInput").ap()
    st_d = nc.dram_tensor("st", [128, CH], f32, kind="ExternalInput").ap()
    wk_d = nc.dram_tensor("wk", [CH, 128, NTAP, F], bf16,
                          kind="ExternalInput").ap()
    # yt[parity][f_half][f][flat ht*WW+w+1]; pad cols stripped on host
    yt_d = nc.dram_tensor("yt", [2, FHN, 128, TLEN], bf16,
                          kind="ExternalOutput").ap()

    with tile.TileContext(nc) as tc:
        with tc.tile_pool(name="pers", bufs=1) as pers, \
             tc.tile_pool(name="xs", bufs=3) as xs, \
             tc.tile_pool(name="wtmp", bufs=1) as wtmp, \
             tc.tile_pool(name="dtmp", bufs=2) as dtmp, \
             tc.tile_pool(name="stage", bufs=3) as stage, \
             tc.tile_pool(name="ps", bufs=7, space="PSUM") as ps, \
             tc.tile_pool(name="psd", bufs=1, space="PSUM") as psd:

            # ---- style scalars ----
            s_t = pers.tile([128, CH], f32, tag="s", name="s_t")
            nc.sync.dma_start(s_t[:], st_d)
            nc.vector.tensor_scalar_add(s_t[:], s_t[:], 1.0)
            hs_t = pers.tile([128, CH], f32, tag="hs", name="hs_t")
            nc.vector.tensor_scalar_mul(hs_t[:], s_t[:], 0.5)

            # ---- raw weights (bf16), DMA'd per ky row, earliest-needed
            # first so group-0 matmuls can start asap ----
            wraw = [pers.tile([128, NTAP, F], bf16, tag=f"wraw{ch}",
                              name=f"wraw{ch}") for ch in range(CH)]
            for ky in (0, 2, 1):
                for ch in range(CH):
                    nc.sync.dma_start(wraw[ch][:, 3 * ky:3 * ky + 3],
                                      wk_d[ch][:, 3 * ky:3 * ky + 3])

            # ---- weight transform: Wt[j][ch] [c, kx, f] bf16 ----
            wt = [[pers.tile([128, 3, F], bf16, tag=f"wt{j}_{ch}",
                             name=f"wt{j}_{ch}") for ch in range(CH)]
                  for j in range(4)]
            for ch in range(CH):
                sc = s_t[:, ch:ch + 1]
                nc.vector.tensor_scalar_mul(wt[0][ch][:], wraw[ch][:, 0:3], sc)
                nc.vector.tensor_scalar_mul(wt[3][ch][:], wraw[ch][:, 6:9], sc)
                su = wtmp.tile([128, 3, F], bf16, tag="su", name="su")
                nc.vector.tensor_add(su[:], wraw[ch][:, 0:3], wraw[ch][:, 6:9])
                sv = wtmp.tile([128, 3, F], bf16, tag="sv", name="sv")
                nc.vector.tensor_add(sv[:], su[:], wraw[ch][:, 3:6])
                nc.vector.tensor_scalar_mul(wt[1][ch][:], sv[:],
                                            hs_t[:, ch:ch + 1])
                sw = wtmp.tile([128, 3, F], bf16, tag="sw", name="sw")
                nc.vector.tensor_sub(sw[:], su[:], wraw[ch][:, 3:6])
                nc.vector.tensor_scalar_mul(wt[2][ch][:], sw[:],
                                            hs_t[:, ch:ch + 1])

            # ---- demod inputs: sq = (raw*s)^2, bf16 (ACT) ----
            sq = [pers.tile([128, NTAP, F], bf16, tag=f"sq{ch}",
                            name=f"sq{ch}") for ch in range(CH)]
            for ch in range(CH):
                nc.scalar.activation(sq[ch][:], wraw[ch][:], AF.Square,
                                     scale=s_t[:, ch:ch + 1])
            ones_t = pers.tile([128, 1], bf16, tag="ones", name="ones_t")
            nc.vector.memset(ones_t[:], 1.0)
            eps_t = pers.tile([128, 1], f32, tag="eps", name="eps_t")
            nc.vector.memset(eps_t[:], 1e-8)
            iv = [pers.tile([128, 1], f32, tag=f"iv{fh}", name=f"iv{fh}")
                  for fh in range(FHN)]
            niv = [pers.tile([128, 1], f32, tag=f"niv{fh}", name=f"niv{fh}")
                   for fh in range(FHN)]

            # ---- T planes: [c, GT + 8320 + GT] bf16, guards zeroed ----
            tp = [[pers.tile([128, GT + TLEN + GT], bf16, tag=f"T{j}_{ch}",
                             name=f"T{j}_{ch}") for ch in range(CH)]
                  for j in range(4)]
            for j in range(4):
                for ch in range(CH):
                    nc.vector.memset(tp[j][ch][:, 0:GT], 0.0)
                    nc.vector.memset(tp[j][ch][:, GT + TLEN:], 0.0)

            # ---- x slot DMA + input transform emission helpers ----
            slot_tiles = {}

            def emit_slot_dma(s):
                if s >= NSLOT or s in slot_tiles:
                    return
                tl = []
                for ch in range(CH):
                    t = xs.tile([128, SROWS, WW], bf16, tag=f"x{ch}",
                                name=f"x{s}_{ch}")
                    nc.sync.dma_start(
                        t[:], xt_d[ch][:, 8 * s * WW:(8 * s + SROWS) * WW])
                    tl.append(t)
                slot_tiles[s] = tl

            done_chunks = set()

            def emit_transform(c):
                if c >= NSLOT or c in done_chunks:
                    return
                done_chunks.add(c)
                emit_slot_dma(c + 3)
                for ch in range(CH):
                    sl = slot_tiles[c][ch]
                    d = [sl[:, k:k + 2 * CHT - 1:2, :] for k in range(4)]
                    o = [tp[j][ch][:, GT + CLEN * c:GT + CLEN * (c + 1)]
                         .rearrange("p (a b) -> p a b", a=CHT)
                         for j in range(4)]
                    nc.vector.tensor_sub(o[0], d[0], d[2])
                    nc.vector.tensor_add(o[1], d[1], d[2])
                    nc.vector.tensor_sub(o[2], d[2], d[1])
                    nc.vector.tensor_sub(o[3], d[1], d[3])

            for s in range(3):
                emit_slot_dma(s)
            emit_transform(0)
            emit_transform(1)

            # ---- main loop ----
            JORD = (0, 3, 1, 2)      # j0/j3 weights are ready earliest
            for g in range(NG):
                npx = 512 if g < NG - 1 else TLEN - 512 * (NG - 1)
                need = min(NSLOT - 1, (512 * (g + 1)) // CLEN)
                for c in range(need + 1):
                    emit_transform(c)

                mt = {}
                for fh in range(FHN):
                    for j in JORD:
                        m = ps.tile([128, 512], f32, tag="m",
                                    name=f"m{j}_{g}_{fh}")
                        mt[j] = m
                        i = 0
                        for kx in range(3):
                            for ch in range(CH):
                                rhs = tp[j][ch][:, GT + 512 * g + kx - 1:
                                                GT + 512 * g + kx - 1 + npx]
                                nc.tensor.matmul(
                                    m[:, :npx],
                                    wt[j][ch][:, kx, fh * 128:(fh + 1) * 128],
                                    rhs, start=(i == 0), stop=(i == 5))
                                i += 1

                    if g == 0 and fh == 0:
                        # demod: d2[f] = sum taps/c of sq; ones-matmul trick.
                        # Emitted after group 0's matmuls: PE stays busy and
                        # invd is ready exactly when group 0 drains.
                        for dfh in range(FHN):
                            d2 = psd.tile([128, 1], f32, tag="d2",
                                          name=f"d2_{dfh}")
                            i = 0
                            for ch in range(CH):
                                for t in range(NTAP):
                                    nc.tensor.matmul(
                                        d2[:],
                                        sq[ch][:, t, dfh * 128:(dfh + 1) * 128],
                                        ones_t[:], start=(i == 0),
                                        stop=(i == CH * NTAP - 1))
                                    i += 1
                            dm = dtmp.tile([128, 1], f32, tag="dm", name="dm")
                            nc.scalar.activation(dm[:], d2[:], AF.Sqrt,
                                                 bias=eps_t[:])
                            nc.vector.reciprocal(iv[dfh][:], dm[:])
                            nc.vector.tensor_scalar_mul(niv[dfh][:],
                                                        iv[dfh][:], -1.0)

                    # ---- drain: output transform fused with demod ----
                    m0, m1, m2, m3 = mt[0], mt[1], mt[2], mt[3]
                    c1s = dtmp.tile([128, 512], f32, tag="c1s", name="c1s")
                    nc.scalar.activation(c1s[:, :npx], m1[:, :npx], AF.Copy,
                                         scale=iv[fh][:])
                    te = dtmp.tile([128, 512], f32, tag="te", name="te")
                    nc.vector.scalar_tensor_tensor(
                        te[:, :npx], m0[:, :npx], iv[fh][:], c1s[:, :npx],
                        ALU.mult, ALU.add)
                    oe = stage.tile([128, 512], bf16, tag="oe", name="oe")
                    nc.vector.scalar_tensor_tensor(
                        oe[:, :npx], m2[:, :npx], iv[fh][:], te[:, :npx],
                        ALU.mult, ALU.add)
                    to = dtmp.tile([128, 512], f32, tag="to", name="to")
                    nc.vector.scalar_tensor_tensor(
                        to[:, :npx], m3[:, :npx], niv[fh][:], c1s[:, :npx],
                        ALU.mult, ALU.add)
                    oo = stage.tile([128, 512], bf16, tag="oo", name="oo")
                    nc.vector.scalar_tensor_tensor(
                        oo[:, :npx], m2[:, :npx], niv[fh][:], to[:, :npx],
                        ALU.mult, ALU.add)
                    nc.gpsimd.dma_start(
                        yt_d[0][fh][:, 512 * g:512 * g + npx], oe[:, :npx])
                    nc.gpsimd.dma_start(
                        yt_d[1][fh][:, 512 * g:512 * g + npx], oo[:, :npx])

    nc.compile()
    return nc


def _get_nc():
    if "nc" not in _COMPILED:
        _COMPILED["nc"] = _build_nc()
    return _COMPILED["nc"]


def _prep_in_maps(x, style, kernel):
    """Host-side layout marshalling: shard over B, transpose+pad+cast x."""
    bf = ml_dtypes.bfloat16
    x = np.ascontiguousarray(x, dtype=np.float32)
    style = np.ascontiguousarray(style, dtype=np.float32)
    kernel = np.ascontiguousarray(kernel, dtype=np.float32)
    # [3,3,C,F] -> [c_half, c_low, tap, f], bf16
    wk = np.ascontiguousarray(
        kernel.reshape(NTAP, CH, 128, F).transpose(1, 2, 0, 3)).astype(bf)
    in_maps = []
    for b in range(B):
        xp = np.zeros((C, HP, WW), dtype=np.float32)
        xp[:, 1:H + 1, 1:W + 1] = x[b].transpose(2, 0, 1)
        xt = np.ascontiguousarray(
            xp.reshape(CH, 128, HP * WW)).astype(bf)
        st = np.ascontiguousarray(style[b].reshape(CH, 128).T)
        in_maps.append({"xt": xt, "st": st, "wk": wk})
    return in_maps


def run_cores(x, style, kernel, trace=False, trace_cores=None):
    """Compile (cached) + run on the 8 NeuronCores. Returns (y, results)."""
    from concourse.bass_utils import run_bass_kernel_spmd

    nc = _get_nc()
    in_maps = _prep_in_maps(x, style, kernel)
    kwargs = {}
    if trace:
        kwargs.update(trace=True, trace_cores=trace_cores)
    res = run_bass_kernel_spmd(nc, in_maps, list(range(N_CORES)), **kwargs)
    y = np.empty((B, H, W, F), dtype=np.float32)
    for b in range(B):
        yt = np.asarray(res.results[b]["yt"]).astype(np.float32)
        # [2, FHN, 128, TLEN] -> strip pad cols, interleave parity rows
        for p in range(2):
            for fh in range(FHN):
                pl = yt[p, fh].reshape(128, NHT, WW)[:, :, 1:W + 1]
                y[b, p::2, :, fh * 128:(fh + 1) * 128] = pl.transpose(1, 2, 0)
    return y, res


def kernel(x, style, kernel):
    y, _ = run_cores(x, style, kernel)
    return y.astype(np.float32)
